# revision 19
# baseline (speedup 1.0000x reference)
"""MultiHeadAttention on 8 trn2 NeuronCores (Bass/Tile SPMD).

Sharding: batch x head-group. Core c handles batch b = c//4 and heads
[4*hg, 4*hg+4) with hg = c%4 (4 of 16 heads, a 256-wide slice of d_model).
Each core computes Q/K (feature-major, [dh, seq]), V (seq-major, [seq, dh]),
scores S^T[L, l] = K_h Q_h^T per head, P^T = exp(S^T/8) (no max subtraction:
scores are O(5), fp32 exp is safe; mask is all-ones by construction),
U^T = [V_h | 1]^T P^T via PSUM accumulation (row 64 = softmax denominator),
C^T = U^T * bcast(1/denom), then the row-sharded out-projection partial
outT = Wo[:, slice] C. Host sums the 4 partials per batch and adds
bo + Wo @ bv (the V-bias commutes through softmax-weighted averaging since
attention rows sum to 1; the K-bias shifts all scores of a query row equally
and cancels in softmax exactly, so it is dropped).

Perf structure:
- All DRAM tensors host-packed to [128, F]; x tensors stream as 8
  [128,2048] descriptors each on the Sync DGE (xk -> xq -> xv) while the
  small weights go concurrently on the Scalar DGE, so the xk transfer
  starts immediately instead of after ~12us of descriptor serialization.
- One-head-lag software pipeline: slot i emits scores for head i
  interleaved with pv for head i-1, so pv never waits on exp (its inputs
  are a full head old) and the in-order PE queue never parks.  The V
  projection (which depends on the last-arriving xv) is emitted in slot 1
  between the two leading score batches.  Softmax drains run two slots
  late; each half's out-projection is slotted into the following half.
- xv reuses xk's SBUF tiles (kproj is done before xv arrives).
- Output DMA'd as bf16; host accumulates in f32.

PSUM (8 banks): "sc" 2 x [128,1024] (scores + out-proj accumulators),
"ps" 4 x [128,512] (qk/v projection transients + two live uacc pairs,
strict round-robin with allocation order arranged deadlock-free).
"""
from contextlib import ExitStack

import numpy as np

import concourse.bacc as bacc
import concourse.bass as bass
import concourse.mybir as mybir
from concourse.bass_utils import run_bass_kernel_spmd
from concourse.tile import TileContext

F32 = mybir.dt.float32
BF16 = mybir.dt.bfloat16
NPBF16 = mybir.dt.np(BF16)
EXPF = mybir.ActivationFunctionType.Exp
ADD = mybir.AluOpType.add
MULT = mybir.AluOpType.mult

SEQ = 2048
DM = 1024
NH = 16  # total heads
HD = 64  # head dim
NCORES = 8
HPC = 4  # heads per core
HB = HPC * HD  # 256-wide head block per core
KT = DM // 128  # 8 contraction tiles
LT = SEQ // 128  # 16 sequence tiles
VW = HD + 1  # 65: V augmented with a ones column per head


def build_nc():
    nc = bacc.Bacc("TRN2", target_bir_lowering=False, debug=False)
    # all inputs host-packed to [128, F] (partition p holds rows {k*128+p})
    xq = nc.declare_dram_parameter("xqT", [128, KT * SEQ], BF16, isOutput=False)
    xk = nc.declare_dram_parameter("xkT", [128, KT * SEQ], BF16, isOutput=False)
    xv = nc.declare_dram_parameter("xvT", [128, KT * SEQ], BF16, isOutput=False)
    wq = nc.declare_dram_parameter("wqT", [128, KT * HB], BF16, isOutput=False)
    wk = nc.declare_dram_parameter("wkT", [128, KT * HB], BF16, isOutput=False)
    wv = nc.declare_dram_parameter("wvT", [128, KT * HB], BF16, isOutput=False)
    wo = nc.declare_dram_parameter("woT", [128, 2 * DM], BF16, isOutput=False)
    bq = nc.declare_dram_parameter("bq", [128, 2], F32, isOutput=False)
    out = nc.declare_dram_parameter("outT", [DM, SEQ], BF16, isOutput=True)

    with TileContext(nc) as tc, ExitStack() as ctx:
        # ---------------- pools ----------------
        pool = lambda name, bufs, **kw: ctx.enter_context(
            tc.tile_pool(name=name, bufs=bufs, **kw)
        )
        consts = pool("consts", 1)
        wpool = pool("weights", 1)
        xpool = pool("x", 1)  # xk/xv share a tag; xq has its own
        qkpool = pool("qk", 1)  # qT/kT persistent [128,2048]x2 each
        vpool = pool("v", LT)  # 16 augmented V tiles
        ctpool = pool("ct", 2)  # per-half C^T, double-buffered across halves
        ptpool = pool("pt", 32)  # two heads of P^T tiles live (1-head lag)
        upool = pool("u", 3)
        dpool = pool("drow", 3)
        rpool = pool("rrow", 3)
        bpool = pool("bcast", 3)
        opool = pool("osb", 3)
        score_ps = pool("score_ps", 2, space="PSUM")  # tag sc: [128,1024]x2
        acc_ps = pool("acc_ps", 4, space="PSUM")  # tag ps: [128,512]x4

        # ones row for the denominator broadcast matmul (K=1)
        ones_sb = consts.tile([1, HD], BF16, tag="ones", name="ones_sb")
        nc.vector.memset(ones_sb[:], 1.0)
        # warm the exp table during the DMA-bound front
        dummy = consts.tile([128, 16], BF16, tag="dummy", name="dummy")
        nc.vector.memset(dummy[:], 0.0)
        nc.scalar.activation(dummy[:], dummy[:], EXPF)

        # -------- DMAs: sync: xk -> xq -> xv; scalar: weights --------
        # Each DMA queue moves a [128,2048] bf16 tile in ~1.8us (~290 GB/s)
        # with depth-4 pipelining, so xk/xq are split across BOTH hardware
        # DGE queues (sync + scalar).  Weights issue first on scalar (tiny).
        # xv goes wholly on sync so the scalar queue is free before the
        # first exp enters the ACT queue.
        def load_w(name, dram, shape):
            t = wpool.tile(shape, BF16, tag=name, name=name)
            nc.scalar.dma_start(t[:], dram[:, :])
            return t

        wk_sb = load_w("wk", wk, [128, KT, HB])
        wq_sb = load_w("wq", wq, [128, KT, HB])
        bq_sb = wpool.tile([128, 2], F32, tag="bq", name="bq")
        nc.scalar.dma_start(bq_sb[:], bq[:, :])
        wv_sb = load_w("wv", wv, [128, KT, HB])
        wo_sb = load_w("wo", wo, [128, 2, DM])

        def xload(tag, dram, split):
            t = xpool.tile([128, KT, SEQ], BF16, tag=tag, name=tag)
            for k in range(KT):
                eng = nc.scalar if (split and k >= KT // 2) else nc.sync
                eng.dma_start(t[:, k, :], dram[:, k * SEQ : (k + 1) * SEQ])
            return t

        xk_sb = xload("xkv", xk, False)
        xq_sb = xload("xq", xq, False)
        # xv reuses xk's SBUF tile (kproj reads complete before xv lands)
        xv_sb = xload("xkv", xv, False)

        # ---------------- phase 1: K then Q projections ----------------
        qT, kT_ = [], []
        for d in range(2):
            qT.append(qkpool.tile([128, SEQ], BF16, tag=f"qT{d}", name=f"qT{d}"))
            kT_.append(qkpool.tile([128, SEQ], BF16, tag=f"kT{d}", name=f"kT{d}"))

        def proj_chain(x_sb, w_sb, dst, bias, d, c):
            ps = acc_ps.tile([128, 512], F32, tag="ps", name="ps")
            for k in range(KT):
                nc.tensor.matmul(
                    ps[:],
                    w_sb[:, k, d * 128 : (d + 1) * 128],
                    x_sb[:, k, c * 512 : (c + 1) * 512],
                    start=(k == 0),
                    stop=(k == KT - 1),
                )
            dstap = dst[d][:, c * 512 : (c + 1) * 512]
            if bias is not None:
                nc.vector.tensor_scalar(dstap, ps[:], bias[:, d : d + 1], None, ADD)
            else:
                nc.vector.tensor_copy(dstap, ps[:])

        for d in range(2):
            for c in range(4):
                proj_chain(xk_sb, wk_sb, kT_, None, d, c)
        for d in range(2):
            for c in range(4):
                proj_chain(xq_sb, wq_sb, qT, bq_sb, d, c)

        # ---------------- V projection (emitted in slot 1) ----------------
        v_sb = []

        def vproj_tile(t):
            vt = vpool.tile([128, HPC * VW], BF16, tag="v", name="vt")
            nc.vector.memset(
                vt[:].rearrange("p (h c) -> p h c", c=VW)[:, :, HD : HD + 1], 1.0
            )
            ps = acc_ps.tile([128, 512], F32, tag="ps", name="ps")
            for k in range(KT):
                nc.tensor.matmul(
                    ps[:, 0:HB],
                    xv_sb[:, k, t * 128 : (t + 1) * 128],
                    wv_sb[:, k, :],
                    start=(k == 0),
                    stop=(k == KT - 1),
                )
            nc.vector.tensor_copy(
                vt[:].rearrange("p (h c) -> p h c", c=VW)[:, :, 0:HD],
                ps[:, 0:HB].rearrange("p (h c) -> p h c", c=HD),
            )
            v_sb.append(vt)

        # ---------------- phase 2: attention ----------------
        ct_tiles = []
        for half in range(2):
            ct_tiles.append(
                [
                    ctpool.tile([128, 1024], BF16, tag=f"ct{d}", name=f"ct{half}{d}")
                    for d in range(2)
                ]
            )

        heads = [(hf, hh) for hf in range(2) for hh in range(HPC)]

        def sc_emit(half, h, t):
            d, r0 = h // 2, (h % 2) * 64
            l0 = half * 1024
            sc = score_ps.tile([128, 1024], F32, tag="sc", name="sc")
            for j in range(2):
                nc.tensor.matmul(
                    sc[:, j * 512 : (j + 1) * 512],
                    kT_[d][r0 : r0 + 64, t * 128 : (t + 1) * 128],
                    qT[d][r0 : r0 + 64, l0 + j * 512 : l0 + (j + 1) * 512],
                    start=True,
                    stop=True,
                )
            pt = ptpool.tile([128, 1024], BF16, tag="pt", name="pt")
            nc.scalar.activation(pt[:], sc[:], EXPF, scale=0.125)
            return pt

        def pv_emit(t, pt, uacc, h):
            for j in range(2):
                nc.tensor.matmul(
                    uacc[j][0:VW, :],
                    v_sb[t][:, h * VW : (h + 1) * VW],
                    pt[:, j * 512 : (j + 1) * 512],
                    start=(t == 0),
                    stop=(t == LT - 1),
                )

        def make_drain(half, h, uacc):
            d, r0 = h // 2, (h % 2) * 64
            ct_d = ct_tiles[half][d]

            def drain():
                for j in range(2):
                    drow = dpool.tile([1, 512], BF16, tag="d", name="drow")
                    with nc.allow_low_precision(reason="softmax denom bcast bf16"):
                        nc.vector.tensor_copy(drow[:], uacc[j][HD : HD + 1, :])
                    usb = upool.tile([64, 512], BF16, tag="u", name="usb")
                    nc.vector.tensor_copy(usb[:], uacc[j][0:HD, :])
                    # broadcast raw denom across 64 partitions (K=1 matmul)
                    # reusing the uacc bank, then reciprocal at full width
                    nc.tensor.matmul(
                        uacc[j][0:HD, :], ones_sb[:], drow[:], start=True, stop=True
                    )
                    rbc = rpool.tile([64, 512], F32, tag="r", name="rbc")
                    nc.vector.reciprocal_approx_fast(rbc[:], uacc[j][0:HD, :])
                    nc.vector.tensor_tensor(
                        ct_d[r0 : r0 + 64, j * 512 : (j + 1) * 512],
                        usb[:],
                        rbc[:],
                        MULT,
                    )

            return drain

        def outproj_chunk(half, ot, copy_engine):
            l0 = half * 1024
            ops = score_ps.tile([128, 1024], F32, tag="sc", name="ops")
            for j in range(2):
                for ci in range(2):
                    nc.tensor.matmul(
                        ops[:, j * 512 : (j + 1) * 512],
                        wo_sb[:, ci, ot * 128 : (ot + 1) * 128],
                        ct_tiles[half][ci][:, j * 512 : (j + 1) * 512],
                        start=(ci == 0),
                        stop=(ci == 1),
                    )
            osb = opool.tile([128, 1024], BF16, tag="osb", name="osb")
            if copy_engine == "scalar":
                nc.scalar.copy(osb[:], ops[:])
            else:
                nc.vector.tensor_copy(osb[:], ops[:])
            nc.sync.dma_start(out[ot * 128 : (ot + 1) * 128, l0 : l0 + 1024], osb[:])

        pts_prev = None  # pt tiles of the previous head
        uacc_prev = None
        drains = []  # pending drain closures (emit 2 slots late)
        pending_outproj = []

        for i, (half, h) in enumerate(heads):
            pts = []
            if i == 0:
                for t in range(LT):
                    pts.append(sc_emit(half, h, t))
            elif i == 1:
                for t in range(LT):
                    pts.append(sc_emit(half, h, t))
                for t in range(LT):
                    vproj_tile(t)
                uacc_prev = [
                    acc_ps.tile([128, 512], F32, tag="ps", name="uacc")
                    for _ in range(2)
                ]
                for t in range(LT):
                    pv_emit(t, pts_prev[t], uacc_prev, heads[0][1])
                drains.append(make_drain(*heads[0], uacc_prev))
            else:
                uacc = [
                    acc_ps.tile([128, 512], F32, tag="ps", name="uacc")
                    for _ in range(2)
                ]
                ph, phh = heads[i - 1]
                last = i == len(heads) - 1
                uacc_l = None
                for t in range(LT):
                    pts.append(sc_emit(half, h, t))
                    if t == 1 and drains:
                        drains.pop(0)()
                    if 2 <= t <= 9 and pending_outproj:
                        eng = "scalar" if t % 2 else "vector"
                        outproj_chunk(*pending_outproj.pop(0), eng)
                    pv_emit(t, pts_prev[t], uacc, phh)
                    if last and t >= 2:
                        # the final head's pv rides in this slot two tiles
                        # behind its exp, so no epilogue pv chain is needed
                        if uacc_l is None:
                            uacc_l = [
                                acc_ps.tile([128, 512], F32, tag="ps", name="uacc")
                                for _ in range(2)
                            ]
                        pv_emit(t - 2, pts[t - 2], uacc_l, h)
                drains.append(make_drain(ph, phh, uacc))
                if phh == HPC - 1:
                    pending_outproj = [(ph, ot) for ot in range(KT)]
            pts_prev = pts

        # epilogue: last two pv tiles, remaining drains, final out-proj
        half, h = heads[-1]
        pv_emit(LT - 2, pts_prev[LT - 2], uacc_l, h)
        pv_emit(LT - 1, pts_prev[LT - 1], uacc_l, h)
        while drains:
            drains.pop(0)()
        drains.append(make_drain(half, h, uacc_l))
        drains.pop(0)()
        for k, ot in enumerate(range(KT)):
            outproj_chunk(1, ot, "scalar" if k % 2 else "vector")

    nc.compile()
    return nc


def _pack128(a, rows):
    # [rows*128, F] -> [128, rows*F] with partition p holding rows {k*128+p}
    f = a.shape[1]
    return np.ascontiguousarray(
        a.reshape(rows, 128, f).transpose(1, 0, 2).reshape(128, rows * f)
    )


def make_in_maps(pre_query, pre_key, pre_value, Wq, bq, Wk, Wv, Wo):
    xt = {}
    for b in range(2):
        for nm, src in (("q", pre_query), ("k", pre_key), ("v", pre_value)):
            xt[(nm, b)] = _pack128(
                np.ascontiguousarray(np.asarray(src)[b].T).astype(NPBF16), KT
            )
    maps = []
    for c in range(NCORES):
        b, hg = c // 4, c % 4
        hs = slice(hg * HB, (hg + 1) * HB)
        maps.append(
            {
                "xqT": xt[("q", b)],
                "xkT": xt[("k", b)],
                "xvT": xt[("v", b)],
                "wqT": _pack128(np.asarray(Wq)[hs, :].T.astype(NPBF16), KT),
                "wkT": _pack128(np.asarray(Wk)[hs, :].T.astype(NPBF16), KT),
                "wvT": _pack128(np.asarray(Wv)[hs, :].T.astype(NPBF16), KT),
                "woT": _pack128(np.asarray(Wo)[:, hs].T.astype(NPBF16), 2),
                "bq": _pack128(
                    np.asarray(bq)[hs].reshape(HB, 1).astype(np.float32), 2
                ),
            }
        )
    return maps


def assemble(results, Wo, bv, bo):
    bias = np.asarray(bo, np.float32) + np.asarray(Wo, np.float32) @ np.asarray(
        bv, np.float32
    )
    out = np.zeros((2, SEQ, DM), np.float32)
    for c in range(NCORES):
        out[c // 4] += results[c]["outT"].astype(np.float32).T
    out += bias[None, None, :]
    return out


def kernel(pre_query, pre_key, pre_value, mask, Wq, bq, Wk, bk, Wv, bv, Wo, bo):
    # mask is all-ones by construction (spec fill=ones); bk cancels in softmax.
    nc = build_nc()
    in_maps = make_in_maps(pre_query, pre_key, pre_value, Wq, bq, Wk, Wv, Wo)
    res = run_bass_kernel_spmd(nc, in_maps, list(range(NCORES)))
    return assemble(res.results, Wo, bv, bo)


# revision 20
# speedup vs baseline: 1.1861x; 1.1861x over previous
"""MultiHeadAttention on 8 trn2 NeuronCores (Bass/Tile SPMD).

Sharding: batch x head-group. Core c handles batch b = c//4 and heads
[4*hg, 4*hg+4) with hg = c%4 (4 of 16 heads, a 256-wide slice of d_model).
Each core computes Q/K (feature-major, [dh, seq]), V (seq-major, [seq, dh]),
scores S^T[L, l] = K_h Q_h^T per head, P^T = exp(S^T/8) (no max subtraction:
scores are O(5), fp32 exp is safe; mask is all-ones by construction),
U^T = [V_h | 1]^T P^T via PSUM accumulation (row 64 = softmax denominator),
C^T = U^T * bcast(1/denom), then the row-sharded out-projection partial
outT = Wo[:, slice] C. Host sums the 4 partials per batch and adds
bo + Wo @ bv (the V-bias commutes through softmax-weighted averaging since
attention rows sum to 1; the K-bias shifts all scores of a query row equally
and cancels in softmax exactly, so it is dropped).

Perf structure:
- All DRAM tensors host-packed to [128, F]; x tensors stream as 8
  [128,2048] descriptors each on the Sync DGE (xk -> xq -> xv) while the
  small weights go concurrently on the Scalar DGE, so the xk transfer
  starts immediately instead of after ~12us of descriptor serialization.
- One-head-lag software pipeline: slot i emits scores for head i
  interleaved with pv for head i-1, so pv never waits on exp (its inputs
  are a full head old) and the in-order PE queue never parks.  The V
  projection (which depends on the last-arriving xv) is emitted in slot 1
  between the two leading score batches.  Softmax drains run two slots
  late; each half's out-projection is slotted into the following half.
- xv reuses xk's SBUF tiles (kproj is done before xv arrives).
- Output DMA'd as bf16; host accumulates in f32.

PSUM (8 banks): "sc" 2 x [128,1024] (scores + out-proj accumulators),
"ps" 4 x [128,512] (qk/v projection transients + two live uacc pairs,
strict round-robin with allocation order arranged deadlock-free).
"""
from contextlib import ExitStack

import numpy as np

import concourse.bacc as bacc
import concourse.bass as bass
import concourse.mybir as mybir
from concourse.bass_utils import run_bass_kernel_spmd
from concourse.tile import TileContext

F32 = mybir.dt.float32
BF16 = mybir.dt.bfloat16
NPBF16 = mybir.dt.np(BF16)
EXPF = mybir.ActivationFunctionType.Exp
ADD = mybir.AluOpType.add
MULT = mybir.AluOpType.mult

SEQ = 2048
DM = 1024
NH = 16  # total heads
HD = 64  # head dim
NCORES = 8
HPC = 4  # heads per core
HB = HPC * HD  # 256-wide head block per core
KT = DM // 128  # 8 contraction tiles
LT = SEQ // 128  # 16 sequence tiles
VW = HD + 1  # 65: V augmented with a ones column per head


def build_nc():
    nc = bacc.Bacc("TRN2", target_bir_lowering=False, debug=False)
    # all inputs host-packed to [128, F] (partition p holds rows {k*128+p})
    xq = nc.declare_dram_parameter("xqT", [128, KT * SEQ], BF16, isOutput=False)
    xk = nc.declare_dram_parameter("xkT", [128, KT * SEQ], BF16, isOutput=False)
    xv = nc.declare_dram_parameter("xvT", [128, KT * SEQ], BF16, isOutput=False)
    wq = nc.declare_dram_parameter("wqT", [128, KT * HB], BF16, isOutput=False)
    wk = nc.declare_dram_parameter("wkT", [128, KT * HB], BF16, isOutput=False)
    wv = nc.declare_dram_parameter("wvT", [128, KT * HB], BF16, isOutput=False)
    wo = nc.declare_dram_parameter("woT", [128, 2 * DM], BF16, isOutput=False)
    bq = nc.declare_dram_parameter("bq", [128, 2], F32, isOutput=False)
    out = nc.declare_dram_parameter("outT", [DM, SEQ], BF16, isOutput=True)

    with TileContext(nc) as tc, ExitStack() as ctx:
        # ---------------- pools ----------------
        pool = lambda name, bufs, **kw: ctx.enter_context(
            tc.tile_pool(name=name, bufs=bufs, **kw)
        )
        consts = pool("consts", 1)
        wpool = pool("weights", 1)
        xpool = pool("x", 1)  # xk/xv share a tag; xq has its own
        qkpool = pool("qk", 1)  # qT/kT persistent [128,2048]x2 each
        vpool = pool("v", LT)  # 16 augmented V tiles
        ctpool = pool("ct", 2)  # per-half C^T, double-buffered across halves
        ptpool = pool("pt", 32)  # two heads of P^T tiles live (1-head lag)
        upool = pool("u", 3)
        dpool = pool("drow", 3)
        rpool = pool("rrow", 3)
        bpool = pool("bcast", 3)
        opool = pool("osb", 4)
        score_ps = pool("score_ps", 2, space="PSUM")  # tag sc: [128,1024]x2
        acc_ps = pool("acc_ps", 4, space="PSUM")  # tag ps: [128,512]x4

        # ones row for the denominator broadcast matmul (K=1)
        ones_sb = consts.tile([1, HD], BF16, tag="ones", name="ones_sb")
        nc.vector.memset(ones_sb[:], 1.0)
        # warm the exp table during the DMA-bound front
        dummy = consts.tile([128, 16], BF16, tag="dummy", name="dummy")
        nc.vector.memset(dummy[:], 0.0)
        nc.scalar.activation(dummy[:], dummy[:], EXPF)

        # -------- DMAs: sync: xk -> xq -> xv; scalar: weights --------
        # Each DMA queue moves a [128,2048] bf16 tile in ~1.8us (~290 GB/s)
        # with depth-4 pipelining, so xk/xq are split across BOTH hardware
        # DGE queues (sync + scalar).  Weights issue first on scalar (tiny).
        # xv goes wholly on sync so the scalar queue is free before the
        # first exp enters the ACT queue.
        def load_w(name, dram, shape):
            t = wpool.tile(shape, BF16, tag=name, name=name)
            nc.scalar.dma_start(t[:], dram[:, :])
            return t

        wk_sb = load_w("wk", wk, [128, KT, HB])
        wq_sb = load_w("wq", wq, [128, KT, HB])
        bq_sb = wpool.tile([128, 2], F32, tag="bq", name="bq")
        nc.scalar.dma_start(bq_sb[:], bq[:, :])
        wv_sb = load_w("wv", wv, [128, KT, HB])
        wo_sb = load_w("wo", wo, [128, 2, DM])

        def xload(tag, dram, split):
            t = xpool.tile([128, KT, SEQ], BF16, tag=tag, name=tag)
            for k in range(KT):
                eng = nc.scalar if (split and k >= KT // 2) else nc.sync
                eng.dma_start(t[:, k, :], dram[:, k * SEQ : (k + 1) * SEQ])
            return t

        xk_sb = xload("xkv", xk, False)
        xq_sb = xload("xq", xq, False)
        # xv reuses xk's SBUF tile (kproj reads complete before xv lands)
        xv_sb = xload("xkv", xv, False)

        # ---------------- phase 1: K then Q projections ----------------
        qT, kT_ = [], []
        for d in range(2):
            qT.append(qkpool.tile([128, SEQ], BF16, tag=f"qT{d}", name=f"qT{d}"))
            kT_.append(qkpool.tile([128, SEQ], BF16, tag=f"kT{d}", name=f"kT{d}"))

        def proj_chain(x_sb, w_sb, dst, bias, d, c):
            ps = acc_ps.tile([128, 512], F32, tag="ps", name="ps")
            for k in range(KT):
                nc.tensor.matmul(
                    ps[:],
                    w_sb[:, k, d * 128 : (d + 1) * 128],
                    x_sb[:, k, c * 512 : (c + 1) * 512],
                    start=(k == 0),
                    stop=(k == KT - 1),
                )
            dstap = dst[d][:, c * 512 : (c + 1) * 512]
            if bias is not None:
                nc.vector.tensor_scalar(dstap, ps[:], bias[:, d : d + 1], None, ADD)
            else:
                nc.vector.tensor_copy(dstap, ps[:])

        for d in range(2):
            for c in range(4):
                proj_chain(xk_sb, wk_sb, kT_, None, d, c)
        for d in range(2):
            for c in range(4):
                proj_chain(xq_sb, wq_sb, qT, bq_sb, d, c)

        # ---------------- V projection (emitted in slot 1) ----------------
        v_sb = []

        def vproj_tile(t):
            vt = vpool.tile([128, HPC * VW], BF16, tag="v", name="vt")
            nc.vector.memset(
                vt[:].rearrange("p (h c) -> p h c", c=VW)[:, :, HD : HD + 1], 1.0
            )
            ps = acc_ps.tile([128, 512], F32, tag="ps", name="ps")
            for k in range(KT):
                nc.tensor.matmul(
                    ps[:, 0:HB],
                    xv_sb[:, k, t * 128 : (t + 1) * 128],
                    wv_sb[:, k, :],
                    start=(k == 0),
                    stop=(k == KT - 1),
                )
            nc.vector.tensor_copy(
                vt[:].rearrange("p (h c) -> p h c", c=VW)[:, :, 0:HD],
                ps[:, 0:HB].rearrange("p (h c) -> p h c", c=HD),
            )
            v_sb.append(vt)

        # ---------------- phase 2: attention ----------------
        ct_tiles = []
        for half in range(2):
            ct_tiles.append(
                [
                    ctpool.tile([128, 1024], BF16, tag=f"ct{d}", name=f"ct{half}{d}")
                    for d in range(2)
                ]
            )

        heads = [(hf, hh) for hf in range(2) for hh in range(HPC)]

        def sc_emit(half, h, t):
            d, r0 = h // 2, (h % 2) * 64
            l0 = half * 1024
            sc = score_ps.tile([128, 1024], F32, tag="sc", name="sc")
            for j in range(2):
                nc.tensor.matmul(
                    sc[:, j * 512 : (j + 1) * 512],
                    kT_[d][r0 : r0 + 64, t * 128 : (t + 1) * 128],
                    qT[d][r0 : r0 + 64, l0 + j * 512 : l0 + (j + 1) * 512],
                    start=True,
                    stop=True,
                )
            pt = ptpool.tile([128, 1024], BF16, tag="pt", name="pt")
            nc.scalar.activation(pt[:], sc[:], EXPF, scale=0.125)
            return pt

        def pv_emit(t, pt, uacc, h):
            for j in range(2):
                nc.tensor.matmul(
                    uacc[j][0:VW, :],
                    v_sb[t][:, h * VW : (h + 1) * VW],
                    pt[:, j * 512 : (j + 1) * 512],
                    start=(t == 0),
                    stop=(t == LT - 1),
                )

        def make_drain(half, h, uacc):
            d, r0 = h // 2, (h % 2) * 64
            ct_d = ct_tiles[half][d]

            def drain():
                for j in range(2):
                    drow = dpool.tile([1, 512], BF16, tag="d", name="drow")
                    with nc.allow_low_precision(reason="softmax denom bcast bf16"):
                        nc.vector.tensor_copy(drow[:], uacc[j][HD : HD + 1, :])
                    usb = upool.tile([64, 512], BF16, tag="u", name="usb")
                    nc.vector.tensor_copy(usb[:], uacc[j][0:HD, :])
                    # broadcast raw denom across 64 partitions (K=1 matmul)
                    # reusing the uacc bank, then reciprocal at full width
                    nc.tensor.matmul(
                        uacc[j][0:HD, :], ones_sb[:], drow[:], start=True, stop=True
                    )
                    rbc = rpool.tile([64, 512], F32, tag="r", name="rbc")
                    nc.vector.reciprocal_approx_fast(rbc[:], uacc[j][0:HD, :])
                    nc.vector.tensor_tensor(
                        ct_d[r0 : r0 + 64, j * 512 : (j + 1) * 512],
                        usb[:],
                        rbc[:],
                        MULT,
                    )

            return drain

        def outproj_chunk(half, ot, copy_engine):
            l0 = half * 1024
            ops = score_ps.tile([128, 1024], F32, tag="sc", name="ops")
            for j in range(2):
                for ci in range(2):
                    nc.tensor.matmul(
                        ops[:, j * 512 : (j + 1) * 512],
                        wo_sb[:, ci, ot * 128 : (ot + 1) * 128],
                        ct_tiles[half][ci][:, j * 512 : (j + 1) * 512],
                        start=(ci == 0),
                        stop=(ci == 1),
                    )
            osb = opool.tile([128, 1024], BF16, tag="osb", name="osb")
            if copy_engine == "scalar":
                nc.scalar.copy(osb[:], ops[:])
            else:
                nc.vector.tensor_copy(osb[:], ops[:])
            nc.sync.dma_start(out[ot * 128 : (ot + 1) * 128, l0 : l0 + 1024], osb[:])

        pts_prev = None  # pt tiles of the previous head
        uacc_prev = None
        drains = []  # pending drain closures (emit 2 slots late)
        pending_outproj = []

        for i, (half, h) in enumerate(heads):
            pts = []
            if i == 0:
                for t in range(LT):
                    pts.append(sc_emit(half, h, t))
            elif i == 1:
                for t in range(LT):
                    pts.append(sc_emit(half, h, t))
                for t in range(LT):
                    vproj_tile(t)
                uacc_prev = [
                    acc_ps.tile([128, 512], F32, tag="ps", name="uacc")
                    for _ in range(2)
                ]
                for t in range(LT):
                    pv_emit(t, pts_prev[t], uacc_prev, heads[0][1])
                drains.append(make_drain(*heads[0], uacc_prev))
            else:
                uacc = [
                    acc_ps.tile([128, 512], F32, tag="ps", name="uacc")
                    for _ in range(2)
                ]
                ph, phh = heads[i - 1]
                last = i == len(heads) - 1
                uacc_l = None
                for t in range(LT):
                    pts.append(sc_emit(half, h, t))
                    if t == 1 and drains:
                        drains.pop(0)()
                    if 2 <= t <= 9 and pending_outproj:
                        eng = "scalar" if t % 2 else "vector"
                        outproj_chunk(*pending_outproj.pop(0), eng)
                    pv_emit(t, pts_prev[t], uacc, phh)
                    if last and t >= 2:
                        # the final head's pv rides in this slot two tiles
                        # behind its exp, so no epilogue pv chain is needed
                        if uacc_l is None:
                            uacc_l = [
                                acc_ps.tile([128, 512], F32, tag="ps", name="uacc")
                                for _ in range(2)
                            ]
                        pv_emit(t - 2, pts[t - 2], uacc_l, h)
                drains.append(make_drain(ph, phh, uacc))
                if phh == HPC - 1:
                    pending_outproj = [(ph, ot) for ot in range(KT)]
            pts_prev = pts

        # epilogue: last two pv tiles, remaining drains, final out-proj
        half, h = heads[-1]
        pv_emit(LT - 2, pts_prev[LT - 2], uacc_l, h)
        pv_emit(LT - 1, pts_prev[LT - 1], uacc_l, h)
        while drains:
            drains.pop(0)()
        drains.append(make_drain(half, h, uacc_l))
        drains.pop(0)()
        # epilogue out-proj: ps banks are free now, so alternate chunks
        # across both psum pools (4-deep rotation) and split each
        # psum->sbuf copy across DVE and ACT so chunks pace at matmul rate
        for ot in range(KT):
            if ot % 2:
                opsj = [
                    acc_ps.tile([128, 512], F32, tag="ps", name="opsf")
                    for _ in range(2)
                ]
                aps = [opsj[0][:], opsj[1][:]]
            else:
                ops = score_ps.tile([128, 1024], F32, tag="sc", name="ops")
                aps = [ops[:, 0:512], ops[:, 512:1024]]
            for j in range(2):
                for ci in range(2):
                    nc.tensor.matmul(
                        aps[j],
                        wo_sb[:, ci, ot * 128 : (ot + 1) * 128],
                        ct_tiles[1][ci][:, j * 512 : (j + 1) * 512],
                        start=(ci == 0),
                        stop=(ci == 1),
                    )
            osb = opool.tile([128, 1024], BF16, tag="osb", name="osb")
            nc.vector.tensor_copy(osb[:, 0:512], aps[0])
            nc.scalar.copy(osb[:, 512:1024], aps[1])
            nc.sync.dma_start(out[ot * 128 : (ot + 1) * 128, 1024:2048], osb[:])

    nc.compile()
    return nc


def _pack128(a, rows):
    # [rows*128, F] -> [128, rows*F] with partition p holding rows {k*128+p}
    f = a.shape[1]
    return np.ascontiguousarray(
        a.reshape(rows, 128, f).transpose(1, 0, 2).reshape(128, rows * f)
    )


def make_in_maps(pre_query, pre_key, pre_value, Wq, bq, Wk, Wv, Wo):
    xt = {}
    for b in range(2):
        for nm, src in (("q", pre_query), ("k", pre_key), ("v", pre_value)):
            xt[(nm, b)] = _pack128(
                np.ascontiguousarray(np.asarray(src)[b].T).astype(NPBF16), KT
            )
    maps = []
    for c in range(NCORES):
        b, hg = c // 4, c % 4
        hs = slice(hg * HB, (hg + 1) * HB)
        maps.append(
            {
                "xqT": xt[("q", b)],
                "xkT": xt[("k", b)],
                "xvT": xt[("v", b)],
                "wqT": _pack128(np.asarray(Wq)[hs, :].T.astype(NPBF16), KT),
                "wkT": _pack128(np.asarray(Wk)[hs, :].T.astype(NPBF16), KT),
                "wvT": _pack128(np.asarray(Wv)[hs, :].T.astype(NPBF16), KT),
                "woT": _pack128(np.asarray(Wo)[:, hs].T.astype(NPBF16), 2),
                "bq": _pack128(
                    np.asarray(bq)[hs].reshape(HB, 1).astype(np.float32), 2
                ),
            }
        )
    return maps


def assemble(results, Wo, bv, bo):
    bias = np.asarray(bo, np.float32) + np.asarray(Wo, np.float32) @ np.asarray(
        bv, np.float32
    )
    out = np.zeros((2, SEQ, DM), np.float32)
    for c in range(NCORES):
        out[c // 4] += results[c]["outT"].astype(np.float32).T
    out += bias[None, None, :]
    return out


def kernel(pre_query, pre_key, pre_value, mask, Wq, bq, Wk, bk, Wv, bv, Wo, bo):
    # mask is all-ones by construction (spec fill=ones); bk cancels in softmax.
    nc = build_nc()
    in_maps = make_in_maps(pre_query, pre_key, pre_value, Wq, bq, Wk, Wv, Wo)
    res = run_bass_kernel_spmd(nc, in_maps, list(range(NCORES)))
    return assemble(res.results, Wo, bv, bo)


# revision 21
# speedup vs baseline: 1.2068x; 1.0175x over previous
"""MultiHeadAttention on 8 trn2 NeuronCores (Bass/Tile SPMD).

Sharding: batch x head-group. Core c handles batch b = c//4 and heads
[4*hg, 4*hg+4) with hg = c%4 (4 of 16 heads, a 256-wide slice of d_model).
Each core computes Q/K (feature-major, [dh, seq]), V (seq-major, [seq, dh]),
scores S^T[L, l] = K_h Q_h^T per head, P^T = exp(S^T/8) (no max subtraction:
scores are O(5), fp32 exp is safe; mask is all-ones by construction),
U^T = [V_h | 1]^T P^T via PSUM accumulation (row 64 = softmax denominator),
C^T = U^T * bcast(1/denom), then the row-sharded out-projection partial
outT = Wo[:, slice] C. Host sums the 4 partials per batch and adds
bo + Wo @ bv (the V-bias commutes through softmax-weighted averaging since
attention rows sum to 1; the K-bias shifts all scores of a query row equally
and cancels in softmax exactly, so it is dropped).

Perf structure:
- All DRAM tensors host-packed to [128, F]; x tensors stream as 8
  [128,2048] descriptors each on the Sync DGE (xk -> xq -> xv) while the
  small weights go concurrently on the Scalar DGE, so the xk transfer
  starts immediately instead of after ~12us of descriptor serialization.
- One-head-lag software pipeline: slot i emits scores for head i
  interleaved with pv for head i-1, so pv never waits on exp (its inputs
  are a full head old) and the in-order PE queue never parks.  The V
  projection (which depends on the last-arriving xv) is emitted in slot 1
  between the two leading score batches.  Softmax drains run two slots
  late; each half's out-projection is slotted into the following half.
- xv reuses xk's SBUF tiles (kproj is done before xv arrives).
- Output DMA'd as bf16; host accumulates in f32.

PSUM (8 banks): "sc" 2 x [128,1024] (scores + out-proj accumulators),
"ps" 4 x [128,512] (qk/v projection transients + two live uacc pairs,
strict round-robin with allocation order arranged deadlock-free).
"""
from contextlib import ExitStack

import numpy as np

import concourse.bacc as bacc
import concourse.bass as bass
import concourse.mybir as mybir
from concourse.bass_utils import run_bass_kernel_spmd
from concourse.tile import TileContext

F32 = mybir.dt.float32
BF16 = mybir.dt.bfloat16
NPBF16 = mybir.dt.np(BF16)
EXPF = mybir.ActivationFunctionType.Exp
ADD = mybir.AluOpType.add
MULT = mybir.AluOpType.mult

SEQ = 2048
DM = 1024
NH = 16  # total heads
HD = 64  # head dim
NCORES = 8
HPC = 4  # heads per core
HB = HPC * HD  # 256-wide head block per core
KT = DM // 128  # 8 contraction tiles
LT = SEQ // 128  # 16 sequence tiles
VW = HD + 1  # 65: V augmented with a ones column per head


def build_nc():
    nc = bacc.Bacc("TRN2", target_bir_lowering=False, debug=False)
    # all inputs host-packed to [128, F] (partition p holds rows {k*128+p})
    xq = nc.declare_dram_parameter("xqT", [128, KT * SEQ], BF16, isOutput=False)
    xk = nc.declare_dram_parameter("xkT", [128, KT * SEQ], BF16, isOutput=False)
    xv = nc.declare_dram_parameter("xvT", [128, KT * SEQ], BF16, isOutput=False)
    wq = nc.declare_dram_parameter("wqT", [128, KT * HB], BF16, isOutput=False)
    wk = nc.declare_dram_parameter("wkT", [128, KT * HB], BF16, isOutput=False)
    wv = nc.declare_dram_parameter("wvT", [128, KT * HB], BF16, isOutput=False)
    wo = nc.declare_dram_parameter("woT", [128, 2 * DM], BF16, isOutput=False)
    bq = nc.declare_dram_parameter("bq", [128, 2], F32, isOutput=False)
    out = nc.declare_dram_parameter("outT", [DM, SEQ], BF16, isOutput=True)

    with TileContext(nc) as tc, ExitStack() as ctx:
        # ---------------- pools ----------------
        pool = lambda name, bufs, **kw: ctx.enter_context(
            tc.tile_pool(name=name, bufs=bufs, **kw)
        )
        consts = pool("consts", 1)
        wpool = pool("weights", 1)
        xpool = pool("x", 1)  # xk/xv share a tag; xq has its own
        qkpool = pool("qk", 1)  # qT/kT persistent [128,2048]x2 each
        vpool = pool("v", LT)  # 16 augmented V tiles
        ctpool = pool("ct", 2)  # per-half C^T, double-buffered across halves
        ptpool = pool("pt", 32)  # two heads of P^T tiles live (1-head lag)
        upool = pool("u", 3)
        dpool = pool("drow", 3)
        rpool = pool("rrow", 3)
        bpool = pool("bcast", 3)
        opool = pool("osb", 4)
        score_ps = pool("score_ps", 2, space="PSUM")  # tag sc: [128,1024]x2
        acc_ps = pool("acc_ps", 4, space="PSUM")  # tag ps: [128,512]x4

        # ones row for the denominator broadcast matmul (K=1)
        ones_sb = consts.tile([1, HD], BF16, tag="ones", name="ones_sb")
        nc.vector.memset(ones_sb[:], 1.0)
        # warm the exp table during the DMA-bound front
        dummy = consts.tile([128, 16], BF16, tag="dummy", name="dummy")
        nc.vector.memset(dummy[:], 0.0)
        nc.scalar.activation(dummy[:], dummy[:], EXPF)

        # -------- DMAs: sync: xk -> xq -> xv; scalar: weights --------
        # Each DMA queue moves a [128,2048] bf16 tile in ~1.8us (~290 GB/s)
        # with depth-4 pipelining, so xk/xq are split across BOTH hardware
        # DGE queues (sync + scalar).  Weights issue first on scalar (tiny).
        # xv goes wholly on sync so the scalar queue is free before the
        # first exp enters the ACT queue.
        def load_w(name, dram, shape):
            t = wpool.tile(shape, BF16, tag=name, name=name)
            nc.scalar.dma_start(t[:], dram[:, :])
            return t

        wk_sb = load_w("wk", wk, [128, KT, HB])
        wq_sb = load_w("wq", wq, [128, KT, HB])
        bq_sb = wpool.tile([128, 2], F32, tag="bq", name="bq")
        nc.scalar.dma_start(bq_sb[:], bq[:, :])
        wv_sb = load_w("wv", wv, [128, KT, HB])
        wo_sb = load_w("wo", wo, [128, 2, DM])

        def xload(tag, dram, split):
            t = xpool.tile([128, KT, SEQ], BF16, tag=tag, name=tag)
            for k in range(KT):
                eng = nc.scalar if (split and k >= KT // 2) else nc.sync
                eng.dma_start(t[:, k, :], dram[:, k * SEQ : (k + 1) * SEQ])
            return t

        xk_sb = xload("xkv", xk, False)
        xq_sb = xload("xq", xq, False)
        # xv reuses xk's SBUF tile (kproj reads complete before xv lands)
        xv_sb = xload("xkv", xv, False)

        # ---------------- phase 1: K then Q projections ----------------
        qT, kT_ = [], []
        for d in range(2):
            qT.append(qkpool.tile([128, SEQ], BF16, tag=f"qT{d}", name=f"qT{d}"))
            kT_.append(qkpool.tile([128, SEQ], BF16, tag=f"kT{d}", name=f"kT{d}"))

        # k-OUTER projections: all 8 chains accumulate in parallel (the
        # whole of PSUM is free during the front: 4 ps banks + 2 sc tiles),
        # so each arriving x tile is consumed immediately instead of chain 0
        # serializing on the last tile while chains 1-7 wait in the queue.
        def proj_all(x_sb, w_sb, dst, bias):
            chains = [(d, c) for d in range(2) for c in range(4)]
            pss = []
            sct = None
            for idx in range(8):
                if idx < 4:
                    ps = acc_ps.tile([128, 512], F32, tag="ps", name="ps")
                    pss.append(ps[:])
                else:
                    if idx % 2 == 0:
                        sct = score_ps.tile([128, 1024], F32, tag="sc", name="psc")
                    pss.append(sct[:, (idx % 2) * 512 : (idx % 2 + 1) * 512])
            for k in range(KT):
                for idx, (d, c) in enumerate(chains):
                    nc.tensor.matmul(
                        pss[idx],
                        w_sb[:, k, d * 128 : (d + 1) * 128],
                        x_sb[:, k, c * 512 : (c + 1) * 512],
                        start=(k == 0),
                        stop=(k == KT - 1),
                    )
            for idx, (d, c) in enumerate(chains):
                dstap = dst[d][:, c * 512 : (c + 1) * 512]
                if bias is not None:
                    nc.vector.tensor_scalar(
                        dstap, pss[idx], bias[:, d : d + 1], None, ADD
                    )
                else:
                    nc.vector.tensor_copy(dstap, pss[idx])

        proj_all(xk_sb, wk_sb, kT_, None)
        proj_all(xq_sb, wq_sb, qT, bq_sb)

        # ---------------- V projection (emitted in slot 1) ----------------
        v_sb = []

        def vproj_tile(t):
            vt = vpool.tile([128, HPC * VW], BF16, tag="v", name="vt")
            nc.vector.memset(
                vt[:].rearrange("p (h c) -> p h c", c=VW)[:, :, HD : HD + 1], 1.0
            )
            ps = acc_ps.tile([128, 512], F32, tag="ps", name="ps")
            for k in range(KT):
                nc.tensor.matmul(
                    ps[:, 0:HB],
                    xv_sb[:, k, t * 128 : (t + 1) * 128],
                    wv_sb[:, k, :],
                    start=(k == 0),
                    stop=(k == KT - 1),
                )
            nc.vector.tensor_copy(
                vt[:].rearrange("p (h c) -> p h c", c=VW)[:, :, 0:HD],
                ps[:, 0:HB].rearrange("p (h c) -> p h c", c=HD),
            )
            v_sb.append(vt)

        # ---------------- phase 2: attention ----------------
        ct_tiles = []
        for half in range(2):
            ct_tiles.append(
                [
                    ctpool.tile([128, 1024], BF16, tag=f"ct{d}", name=f"ct{half}{d}")
                    for d in range(2)
                ]
            )

        heads = [(hf, hh) for hf in range(2) for hh in range(HPC)]

        def sc_emit(half, h, t):
            d, r0 = h // 2, (h % 2) * 64
            l0 = half * 1024
            sc = score_ps.tile([128, 1024], F32, tag="sc", name="sc")
            for j in range(2):
                nc.tensor.matmul(
                    sc[:, j * 512 : (j + 1) * 512],
                    kT_[d][r0 : r0 + 64, t * 128 : (t + 1) * 128],
                    qT[d][r0 : r0 + 64, l0 + j * 512 : l0 + (j + 1) * 512],
                    start=True,
                    stop=True,
                )
            pt = ptpool.tile([128, 1024], BF16, tag="pt", name="pt")
            nc.scalar.activation(pt[:], sc[:], EXPF, scale=0.125)
            return pt

        def pv_emit(t, pt, uacc, h):
            for j in range(2):
                nc.tensor.matmul(
                    uacc[j][0:VW, :],
                    v_sb[t][:, h * VW : (h + 1) * VW],
                    pt[:, j * 512 : (j + 1) * 512],
                    start=(t == 0),
                    stop=(t == LT - 1),
                )

        def make_drain(half, h, uacc):
            d, r0 = h // 2, (h % 2) * 64
            ct_d = ct_tiles[half][d]

            def drain():
                for j in range(2):
                    drow = dpool.tile([1, 512], BF16, tag="d", name="drow")
                    with nc.allow_low_precision(reason="softmax denom bcast bf16"):
                        nc.vector.tensor_copy(drow[:], uacc[j][HD : HD + 1, :])
                    usb = upool.tile([64, 512], BF16, tag="u", name="usb")
                    nc.vector.tensor_copy(usb[:], uacc[j][0:HD, :])
                    # broadcast raw denom across 64 partitions (K=1 matmul)
                    # reusing the uacc bank, then reciprocal at full width
                    nc.tensor.matmul(
                        uacc[j][0:HD, :], ones_sb[:], drow[:], start=True, stop=True
                    )
                    rbc = rpool.tile([64, 512], F32, tag="r", name="rbc")
                    nc.vector.reciprocal_approx_fast(rbc[:], uacc[j][0:HD, :])
                    nc.vector.tensor_tensor(
                        ct_d[r0 : r0 + 64, j * 512 : (j + 1) * 512],
                        usb[:],
                        rbc[:],
                        MULT,
                    )

            return drain

        def outproj_chunk(half, ot, copy_engine):
            l0 = half * 1024
            ops = score_ps.tile([128, 1024], F32, tag="sc", name="ops")
            for j in range(2):
                for ci in range(2):
                    nc.tensor.matmul(
                        ops[:, j * 512 : (j + 1) * 512],
                        wo_sb[:, ci, ot * 128 : (ot + 1) * 128],
                        ct_tiles[half][ci][:, j * 512 : (j + 1) * 512],
                        start=(ci == 0),
                        stop=(ci == 1),
                    )
            osb = opool.tile([128, 1024], BF16, tag="osb", name="osb")
            if copy_engine == "scalar":
                nc.scalar.copy(osb[:], ops[:])
            else:
                nc.vector.tensor_copy(osb[:], ops[:])
            nc.sync.dma_start(out[ot * 128 : (ot + 1) * 128, l0 : l0 + 1024], osb[:])

        pts_prev = None  # pt tiles of the previous head
        uacc_prev = None
        drains = []  # pending drain closures (emit 2 slots late)
        pending_outproj = []

        for i, (half, h) in enumerate(heads):
            pts = []
            if i == 0:
                for t in range(LT):
                    pts.append(sc_emit(half, h, t))
            elif i == 1:
                for t in range(LT):
                    pts.append(sc_emit(half, h, t))
                for t in range(LT):
                    vproj_tile(t)
                uacc_prev = [
                    acc_ps.tile([128, 512], F32, tag="ps", name="uacc")
                    for _ in range(2)
                ]
                for t in range(LT):
                    pv_emit(t, pts_prev[t], uacc_prev, heads[0][1])
                drains.append(make_drain(*heads[0], uacc_prev))
            else:
                uacc = [
                    acc_ps.tile([128, 512], F32, tag="ps", name="uacc")
                    for _ in range(2)
                ]
                ph, phh = heads[i - 1]
                last = i == len(heads) - 1
                uacc_l = None
                for t in range(LT):
                    pts.append(sc_emit(half, h, t))
                    if t == 1 and drains:
                        drains.pop(0)()
                    if 2 <= t <= 9 and pending_outproj:
                        eng = "scalar" if t % 2 else "vector"
                        outproj_chunk(*pending_outproj.pop(0), eng)
                    pv_emit(t, pts_prev[t], uacc, phh)
                    if last and t >= 2:
                        # the final head's pv rides in this slot two tiles
                        # behind its exp, so no epilogue pv chain is needed
                        if uacc_l is None:
                            uacc_l = [
                                acc_ps.tile([128, 512], F32, tag="ps", name="uacc")
                                for _ in range(2)
                            ]
                        pv_emit(t - 2, pts[t - 2], uacc_l, h)
                drains.append(make_drain(ph, phh, uacc))
                if phh == HPC - 1:
                    pending_outproj = [(ph, ot) for ot in range(KT)]
            pts_prev = pts

        # epilogue: last two pv tiles, remaining drains, final out-proj
        half, h = heads[-1]
        pv_emit(LT - 2, pts_prev[LT - 2], uacc_l, h)
        pv_emit(LT - 1, pts_prev[LT - 1], uacc_l, h)
        while drains:
            drains.pop(0)()
        drains.append(make_drain(half, h, uacc_l))
        drains.pop(0)()
        # epilogue out-proj: ps banks are free now, so alternate chunks
        # across both psum pools (4-deep rotation) and split each
        # psum->sbuf copy across DVE and ACT so chunks pace at matmul rate
        for ot in range(KT):
            if ot % 2:
                opsj = [
                    acc_ps.tile([128, 512], F32, tag="ps", name="opsf")
                    for _ in range(2)
                ]
                aps = [opsj[0][:], opsj[1][:]]
            else:
                ops = score_ps.tile([128, 1024], F32, tag="sc", name="ops")
                aps = [ops[:, 0:512], ops[:, 512:1024]]
            for j in range(2):
                for ci in range(2):
                    nc.tensor.matmul(
                        aps[j],
                        wo_sb[:, ci, ot * 128 : (ot + 1) * 128],
                        ct_tiles[1][ci][:, j * 512 : (j + 1) * 512],
                        start=(ci == 0),
                        stop=(ci == 1),
                    )
            osb = opool.tile([128, 1024], BF16, tag="osb", name="osb")
            nc.vector.tensor_copy(osb[:, 0:512], aps[0])
            nc.scalar.copy(osb[:, 512:1024], aps[1])
            nc.sync.dma_start(out[ot * 128 : (ot + 1) * 128, 1024:2048], osb[:])

    nc.compile()
    return nc


def _pack128(a, rows):
    # [rows*128, F] -> [128, rows*F] with partition p holding rows {k*128+p}
    f = a.shape[1]
    return np.ascontiguousarray(
        a.reshape(rows, 128, f).transpose(1, 0, 2).reshape(128, rows * f)
    )


def make_in_maps(pre_query, pre_key, pre_value, Wq, bq, Wk, Wv, Wo):
    xt = {}
    for b in range(2):
        for nm, src in (("q", pre_query), ("k", pre_key), ("v", pre_value)):
            xt[(nm, b)] = _pack128(
                np.ascontiguousarray(np.asarray(src)[b].T).astype(NPBF16), KT
            )
    maps = []
    for c in range(NCORES):
        b, hg = c // 4, c % 4
        hs = slice(hg * HB, (hg + 1) * HB)
        maps.append(
            {
                "xqT": xt[("q", b)],
                "xkT": xt[("k", b)],
                "xvT": xt[("v", b)],
                "wqT": _pack128(np.asarray(Wq)[hs, :].T.astype(NPBF16), KT),
                "wkT": _pack128(np.asarray(Wk)[hs, :].T.astype(NPBF16), KT),
                "wvT": _pack128(np.asarray(Wv)[hs, :].T.astype(NPBF16), KT),
                "woT": _pack128(np.asarray(Wo)[:, hs].T.astype(NPBF16), 2),
                "bq": _pack128(
                    np.asarray(bq)[hs].reshape(HB, 1).astype(np.float32), 2
                ),
            }
        )
    return maps


def assemble(results, Wo, bv, bo):
    bias = np.asarray(bo, np.float32) + np.asarray(Wo, np.float32) @ np.asarray(
        bv, np.float32
    )
    out = np.zeros((2, SEQ, DM), np.float32)
    for c in range(NCORES):
        out[c // 4] += results[c]["outT"].astype(np.float32).T
    out += bias[None, None, :]
    return out


def kernel(pre_query, pre_key, pre_value, mask, Wq, bq, Wk, bk, Wv, bv, Wo, bo):
    # mask is all-ones by construction (spec fill=ones); bk cancels in softmax.
    nc = build_nc()
    in_maps = make_in_maps(pre_query, pre_key, pre_value, Wq, bq, Wk, Wv, Wo)
    res = run_bass_kernel_spmd(nc, in_maps, list(range(NCORES)))
    return assemble(res.results, Wo, bv, bo)
